# revision 16
# baseline (speedup 1.0000x reference)
"""GATv2 3-layer GNN on 8 Trainium2 NeuronCores (Bass/Tile).

Strategy (vertex-cut by destination):
  - Nodes padded to NP = NCORES*NBLK*128. Core r owns dst nodes
    [r*PER, (r+1)*PER), PER = NBLK*128.
  - Edges bucketed by (core, 128-node dst block). Within a block, edges are
    split by src < HALF (lo) / src >= HALF (hi) so gather indices fit int16,
    each side padded to a uniform chunk count (C_lo / C_hi chunks of 128).
  - Per layer: dense phase computes fs = h@Ws.T+bs for ALL nodes (replicated)
    and fd|res for owned nodes; edge phase gathers fs[src], fd[dst] rows with
    batched dma_gather, computes scores with DVE/ACT ops, and aggregates with
    one-hot matmuls into PSUM (z accumulated alongside as extra columns).
    Softmax normalization is folded to the node side: rst = (sum exp*fs)/z.
    Scores are tiny (|s| < 2) so no max-subtraction is needed.
  - Between layers: AllGather of the owned feature-major h slice.
"""

import math
import time
from contextlib import ExitStack

import numpy as np

try:
    import concourse  # noqa: F401
except ImportError:  # pragma: no cover
    import sys

    sys.path.insert(0, "/opt/trn_rl_repo")

import concourse.bacc as bacc
import concourse.mybir as mybir
import concourse.tile as tile
from concourse.bass_utils import run_bass_kernel_spmd

P = 128
NCORES = 8
H = 4
SLOPE = 0.2
F32 = mybir.dt.float32
I16 = mybir.dt.int16
AF = mybir.ActivationFunctionType
ALU = mybir.AluOpType


# ----------------------------------------------------------------- host prep

def _prep(x, src, dst, params):
    x = np.asarray(x, np.float32)
    src = np.asarray(src, np.int64)
    dst = np.asarray(dst, np.int64)
    N, IN = x.shape
    assert IN == P
    f_outs = [int(np.asarray(p["Ws"]).shape[0]) // H for p in params]
    Fs = [H * f for f in f_outs]          # 128, 128, 188
    FPs = [(F + 15) // 16 * 16 for F in Fs]  # 128, 128, 192
    for F, FP in zip(Fs, FPs):
        assert FP - F in (0, 4), (F, FP)

    NBLK = math.ceil(N / (NCORES * P))
    PER = NBLK * P
    NP = NCORES * PER
    assert NP <= 2 * 32768
    HALF = NP // 2
    assert HALF <= 32768

    core = dst // PER
    blk = (dst % PER) // P
    dloc = dst % P
    is_lo = src < HALF

    # bucket edges per (core, blk, lo/hi)
    buckets = {}
    for flag in (True, False):
        sel = is_lo == flag
        key = (core[sel] * NBLK + blk[sel]).astype(np.int64)
        order = np.argsort(key, kind="stable")
        ks = key[order]
        srcs = src[sel][order] - (0 if flag else HALF)
        dls = dloc[sel][order]
        starts = np.searchsorted(ks, np.arange(NCORES * NBLK))
        ends = np.searchsorted(ks, np.arange(NCORES * NBLK), side="right")
        buckets[flag] = (srcs, dls, starts, ends)

    n_lo = (buckets[True][3] - buckets[True][2]).max()
    n_hi = (buckets[False][3] - buckets[False][2]).max()
    C_lo = max(1, math.ceil(int(n_lo) / P))
    C_hi = max(1, math.ceil(int(n_hi) / P))
    C = C_lo + C_hi

    idx_lo = np.zeros((NCORES, NBLK, C_lo * P), np.int16)
    idx_hi = np.zeros((NCORES, NBLK, C_hi * P), np.int16)
    idx_fd = np.zeros((NCORES, NBLK, C * P), np.int16)
    dstl = np.full((NCORES, NBLK, C * P), 200.0, np.float32)

    for flag, idx_arr, off, CC in ((True, idx_lo, 0, C_lo), (False, idx_hi, C_lo * P, C_hi)):
        srcs, dls, starts, ends = buckets[flag]
        for r in range(NCORES):
            for b in range(NBLK):
                s, e = starts[r * NBLK + b], ends[r * NBLK + b]
                n = e - s
                idx_arr[r, b, :n] = srcs[s:e].astype(np.int16)
                dstl[r, b, off:off + n] = dls[s:e].astype(np.float32)
                idx_fd[r, b, off:off + n] = (b * P + dls[s:e]).astype(np.int16)

    def wrap16(a):
        # dma_gather idx layout: index j -> [partition j%16 (replicated x8), j//16]
        ncr, nb, M = a.shape
        w = a.reshape(ncr, nb, M // 16, 16).transpose(0, 1, 3, 2)  # [.., 16, W]
        return np.tile(w, (1, 1, 8, 1)).copy()  # [.., 128, W]

    idx_lo_w = wrap16(idx_lo)
    idx_hi_w = wrap16(idx_hi)
    idx_fd_w = wrap16(idx_fd)
    dstl_a = dstl.reshape(NCORES, NBLK, C, P).transpose(0, 1, 3, 2).copy()

    x_fm = np.zeros((P, NP), np.float32)
    x_fm[:, :N] = x.T

    shared = {"x_fm": x_fm,
              "iota": np.tile(np.arange(P, dtype=np.float32), (P, 1)),
              "ident": np.eye(P, dtype=np.float32)}
    for l, p in enumerate(params):
        F, FP = Fs[l], FPs[l]
        Ws = np.asarray(p["Ws"], np.float32)
        Wd = np.asarray(p["Wd"], np.float32)
        Wr = np.asarray(p["Wr"], np.float32)
        bs = np.asarray(p["bs"], np.float32)
        bd = np.asarray(p["bd"], np.float32)
        br = np.asarray(p["br"], np.float32)
        attn = np.asarray(p["attn"], np.float32).reshape(-1)
        wsT = np.zeros((P, FP), np.float32)
        wsT[:, :F] = Ws.T
        wdrT = np.zeros((P, 2 * FP), np.float32)
        wdrT[:, :F] = Wd.T
        wdrT[:, FP:FP + F] = Wr.T
        attn_b = np.zeros((P, FP), np.float32)
        attn_b[:, :F] = attn
        gbps = 512 // FP
        bs_pad = np.zeros(FP, np.float32)
        bs_pad[:F] = bs
        bs4 = np.tile(bs_pad, gbps)[None, :]
        gbps_f = max(1, 512 // (2 * FP))
        bdr = np.zeros(2 * FP, np.float32)
        bdr[:F] = bd
        bdr[FP:FP + F] = br
        bdr2 = np.tile(bdr, gbps_f)[None, :]
        shared.update({f"wsT{l}": wsT, f"wdrT{l}": wdrT, f"attn{l}": attn_b,
                       f"bs4_{l}": bs4, f"bdr2_{l}": bdr2})

    per_core = []
    for r in range(NCORES):
        per_core.append({
            "x_own": x_fm[:, r * PER:(r + 1) * PER].copy(),
            "idx_lo": idx_lo_w[r], "idx_hi": idx_hi_w[r],
            "idx_fd": idx_fd_w[r], "dstl": dstl_a[r],
        })

    meta = dict(N=N, NP=NP, PER=PER, NBLK=NBLK, HALF=HALF,
                C_lo=C_lo, C_hi=C_hi, C=C, Fs=tuple(Fs), FPs=tuple(FPs),
                f_outs=tuple(f_outs))
    return meta, shared, per_core


# ------------------------------------------------------------- program build

def _build(meta):
    NP, PER, NBLK = meta["NP"], meta["PER"], meta["NBLK"]
    HALF, C_lo, C_hi, C = meta["HALF"], meta["C_lo"], meta["C_hi"], meta["C"]
    Fs, FPs, f_outs = meta["Fs"], meta["FPs"], meta["f_outs"]
    NBLKG = NP // P
    NC_out = f_outs[2]

    # dynamic_dma_scratch_size sets the SWDGE descriptor-ring capacity
    # (ring holds size//16 descriptors; a dma_gather of num_idxs rows needs
    # num_idxs+1 slots or its decode-side await_space deadlocks).
    max_idxs = max(C_lo, C_hi, C) * P
    scratch = max(16384, ((max_idxs + 1) * 16 + 16383) // 16384 * 16384)
    nc = bacc.Bacc("TRN2", target_bir_lowering=False, debug=False,
                   num_devices=NCORES, dynamic_dma_scratch_size=scratch)

    # ---- I/O
    inp = {}
    for name, shape in [("x_fm", [P, NP]), ("x_own", [P, PER]),
                        ("iota", [P, P]), ("ident", [P, P])]:
        inp[name] = nc.dram_tensor(name, shape, F32, kind="ExternalInput")
    for l in range(3):
        FP = FPs[l]
        gbps = 512 // FP
        gbps_f = max(1, 512 // (2 * FP))
        for name, shape in [(f"wsT{l}", [P, FP]), (f"wdrT{l}", [P, 2 * FP]),
                            (f"attn{l}", [P, FP]),
                            (f"bs4_{l}", [1, gbps * FP]),
                            (f"bdr2_{l}", [1, gbps_f * 2 * FP])]:
            inp[name] = nc.dram_tensor(name, shape, F32, kind="ExternalInput")
    inp["idx_lo"] = nc.dram_tensor("idx_lo", [NBLK, P, C_lo * 8], I16,
                                   kind="ExternalInput")
    inp["idx_hi"] = nc.dram_tensor("idx_hi", [NBLK, P, C_hi * 8], I16,
                                   kind="ExternalInput")
    inp["idx_fd"] = nc.dram_tensor("idx_fd", [NBLK, P, C * 8], I16,
                                   kind="ExternalInput")
    inp["dstl"] = nc.dram_tensor("dstl", [NBLK, P, C], F32,
                                 kind="ExternalInput")
    out_t = nc.dram_tensor("out", [PER, NC_out], F32, kind="ExternalOutput")

    with tile.TileContext(nc) as tc, ExitStack() as ctx:
        cst = ctx.enter_context(tc.tile_pool(name="cst", bufs=1))
        dram = ctx.enter_context(tc.tile_pool(name="dram", bufs=1, space="DRAM"))
        dense = ctx.enter_context(tc.tile_pool(name="dense", bufs=2))
        edge = ctx.enter_context(tc.tile_pool(name="edge", bufs=2))
        inter = ctx.enter_context(tc.tile_pool(name="inter", bufs=1))
        psum = ctx.enter_context(tc.tile_pool(name="psum", bufs=2, space="PSUM"))

        # ---- constants to SBUF
        def cload(name, shape, dt=F32):
            t = cst.tile(shape, dt, tag=name)
            nc.sync.dma_start(t[:], inp[name][:])
            return t

        iota_t = cload("iota", [P, P])
        ident_t = cload("ident", [P, P])
        ones_t = cst.tile([1, P], F32, tag="ones")
        nc.vector.memset(ones_t[:], 1.0)
        wsT_t, wdrT_t, attn_t, bs4_t, bdr2_t = [], [], [], [], []
        for l in range(3):
            FP = FPs[l]
            gbps = 512 // FP
            gbps_f = max(1, 512 // (2 * FP))
            wsT_t.append(cload(f"wsT{l}", [P, FP]))
            wdrT_t.append(cload(f"wdrT{l}", [P, 2 * FP]))
            attn_t.append(cload(f"attn{l}", [P, FP]))
            bs4_t.append(cload(f"bs4_{l}", [1, gbps * FP]))
            bdr2_t.append(cload(f"bdr2_{l}", [1, gbps_f * 2 * FP]))

        # ---- internal DRAM
        fs_d, fdr_d, hown_d, hag_d = [], [], [], []
        for l in range(3):
            FP = FPs[l]
            fs_d.append(dram.tile([NP, FP], F32, tag=f"fs{l}", name=f"fs{l}"))
            fdr_d.append(dram.tile([PER, 2 * FP], F32, tag=f"fdr{l}", name=f"fdr{l}"))
            if l < 2:
                hown_d.append(dram.tile([P, PER], F32, tag=f"hown{l}", name=f"hown{l}"))
                hag_d.append(dram.tile([NCORES, P, PER], F32, tag=f"hag{l}",
                                       name=f"hag{l}", addr_space="Shared"))

        def dense_pass(src_aps, w_t, b_t, width, stage_tag, store_fn):
            """src_aps: list of (dram_ap, nblocks). Computes, for each 128-node
            block, psum = bias + h_blk @ W and stores via store_fn(stage, i0, nb)."""
            gb = 512 // width  # blocks per psum tile
            for i0, (ap, nb) in src_aps:
                ht = dense.tile([P, nb * P], F32, tag="ht")
                nc.sync.dma_start(ht[:], ap)
                stage = dense.tile([P, nb, width], F32, tag=stage_tag)
                for g0 in range(0, nb, gb):
                    gn = min(gb, nb - g0)
                    ps = psum.tile([P, 512], F32, tag="ps_dense")
                    nc.tensor.matmul(ps[:, :gn * width], lhsT=ones_t[:, :P],
                                     rhs=b_t[:, :gn * width],
                                     start=True, stop=False,
                                     skip_group_check=True)
                    for j in range(gn):
                        nc.tensor.matmul(
                            ps[:, j * width:(j + 1) * width],
                            lhsT=ht[:, (g0 + j) * P:(g0 + j + 1) * P],
                            rhs=w_t[:],
                            start=False, stop=(j == gn - 1),
                            skip_group_check=True)
                    nc.vector.tensor_copy(
                        out=stage[:, g0:g0 + gn, :].rearrange("p b f -> p (b f)"),
                        in_=ps[:, :gn * width])
                store_fn(stage, i0, nb)

        def batches(total, bmax):
            o = 0
            while o < total:
                n = min(bmax, total - o)
                yield o, n
                o += n

        for l in range(3):
            F, FP, Fh = Fs[l], FPs[l], f_outs[l]
            AGW = F + 4

            # ---------- dense: fs over ALL nodes
            if l == 0:
                src_aps = [((o, (inp["x_fm"][:, o * P:(o + n) * P], n)))
                           for o, n in batches(NBLKG, 4)]
            else:
                src_aps = []
                for r in range(NCORES):
                    for o, n in batches(NBLK, 4):
                        src_aps.append((r * NBLK + o,
                                        (hag_d[l - 1][r, :, o * P:(o + n) * P], n)))
            fsl = fs_d[l]

            def store_fs(stage, i0, nb, fsl=fsl, FP=FP):
                view = fsl[i0 * P:(i0 + nb) * P, :].rearrange(
                    "(b p) f -> p b f", p=P)
                nc.sync.dma_start(view, stage[:, :nb, :])

            dense_pass(src_aps, wsT_t[l], bs4_t[l], FP, "stage", store_fs)

            # ---------- dense: fd|res over OWN nodes
            own_src = inp["x_own"] if l == 0 else hown_d[l - 1]
            src_aps = [((o, (own_src[:, o * P:(o + n) * P], n)))
                       for o, n in batches(NBLK, 4)]
            fdrl = fdr_d[l]

            def store_fdr(stage, i0, nb, fdrl=fdrl, FP=FP):
                view = fdrl[i0 * P:(i0 + nb) * P, :].rearrange(
                    "(b p) f -> p b f", p=P)
                nc.sync.dma_start(view, stage[:, :nb, :])

            dense_pass(src_aps, wdrT_t[l], bdr2_t[l], 2 * FP, "stage",
                       store_fdr)

            # ---------- edge phase
            for b in range(NBLK):
                ilo = edge.tile([P, C_lo * 8], I16, tag="ilo")
                nc.sync.dma_start(ilo[:], inp["idx_lo"][b])
                ihi = edge.tile([P, C_hi * 8], I16, tag="ihi")
                nc.sync.dma_start(ihi[:], inp["idx_hi"][b])
                ifd = edge.tile([P, C * 8], I16, tag="ifd")
                nc.sync.dma_start(ifd[:], inp["idx_fd"][b])
                dstl_t = edge.tile([P, C], F32, tag="dstl")
                nc.sync.dma_start(dstl_t[:], inp["dstl"][b])
                res_t = edge.tile([P, FP], F32, tag="res")
                nc.sync.dma_start(res_t[:], fdrl[b * P:(b + 1) * P, FP:])

                fsg = edge.tile([P, C, FP], F32, tag="fsg")
                nc.gpsimd.dma_gather(
                    fsg[:, :C_lo, :], fsl[:, :], ilo[:],
                    num_idxs=C_lo * P, num_idxs_reg=C_lo * P,
                    elem_size=FP, single_packet=False)
                nc.gpsimd.dma_gather(
                    fsg[:, C_lo:, :], fsl[HALF:, :], ihi[:],
                    num_idxs=C_hi * P, num_idxs_reg=C_hi * P,
                    elem_size=FP, single_packet=False)
                fdg = edge.tile([P, C, FP], F32, tag="fdg")
                nc.gpsimd.dma_gather(
                    fdg[:, :, :], fdrl[:, :FP], ifd[:],
                    num_idxs=C * P, num_idxs_reg=C * P,
                    elem_size=FP, elem_step=2 * FP, single_packet=False)

                oh = inter.tile([P, C, P], F32, tag="oh")
                nc.vector.tensor_tensor(
                    out=oh[:],
                    in0=dstl_t[:, :, None].to_broadcast([P, C, P]),
                    in1=iota_t[:, None, :].to_broadcast([P, C, P]),
                    op=ALU.is_equal)
                q = inter.tile([P, C, FP], F32, tag="q")
                nc.vector.tensor_add(out=q[:], in0=fsg[:], in1=fdg[:])
                lr = inter.tile([P, C, FP], F32, tag="lr")
                nc.scalar.activation(lr[:], q[:], AF.Prelu, alpha=SLOPE)
                sm = inter.tile([P, C, FP], F32, tag="sm")
                nc.vector.tensor_mul(
                    out=sm[:], in0=lr[:],
                    in1=attn_t[l][:, None, :].to_broadcast([P, C, FP]))
                score = inter.tile([P, C, H], F32, tag="score")
                nc.vector.reduce_sum(
                    out=score[:],
                    in_=sm[:, :, :F].rearrange("p c (h f) -> p c h f", h=H),
                    axis=mybir.AxisListType.X)
                agg = inter.tile([P, C, AGW], F32, tag="agg")
                nc.scalar.activation(agg[:, :, F:], score[:], AF.Exp)
                nc.vector.tensor_mul(
                    out=agg[:, :, :F].rearrange("p c (h f) -> p c h f", h=H),
                    in0=fsg[:, :, :F].rearrange("p c (h f) -> p c h f", h=H),
                    in1=agg[:, :, F:][:, :, :, None].to_broadcast([P, C, H, Fh]))

                ps_ag = psum.tile([P, AGW], F32, tag="ps_ag")
                for c in range(C):
                    nc.tensor.matmul(ps_ag[:], lhsT=oh[:, c, :],
                                     rhs=agg[:, c, :],
                                     start=(c == 0), stop=(c == C - 1))

                zr = inter.tile([P, H], F32, tag="zr")
                nc.vector.tensor_scalar_max(out=zr[:], in0=ps_ag[:, F:],
                                            scalar1=1e-30)
                zri = inter.tile([P, H], F32, tag="zri")
                nc.vector.reciprocal(out=zri[:], in_=zr[:])

                if l < 2:
                    hb = inter.tile([P, F], F32, tag="hb")
                    nc.vector.tensor_mul(
                        out=hb[:].rearrange("p (h f) -> p h f", h=H),
                        in0=ps_ag[:, :F].rearrange("p (h f) -> p h f", h=H),
                        in1=zri[:, :, None].to_broadcast([P, H, Fh]))
                    hb2 = inter.tile([P, F], F32, tag="hb2")
                    nc.vector.tensor_add(out=hb2[:], in0=hb[:],
                                         in1=res_t[:, :F])
                    pst = psum.tile([P, P], F32, tag="pst")
                    nc.tensor.transpose(pst[:], hb2[:], ident_t[:])
                    hfmb = inter.tile([P, P], F32, tag="hfmb")
                    nc.vector.tensor_copy(out=hfmb[:], in_=pst[:])
                    nc.sync.dma_start(hown_d[l][:, b * P:(b + 1) * P],
                                      hfmb[:])
                else:
                    rr = inter.tile([P, F], F32, tag="hb")
                    nc.vector.tensor_mul(
                        out=rr[:].rearrange("p (h f) -> p h f", h=H),
                        in0=ps_ag[:, :F].rearrange("p (h f) -> p h f", h=H),
                        in1=zri[:, :, None].to_broadcast([P, H, Fh]))
                    rr2 = inter.tile([P, F], F32, tag="hb2")
                    nc.vector.tensor_add(out=rr2[:], in0=rr[:],
                                         in1=res_t[:, :F])
                    r4 = rr2[:].rearrange("p (h f) -> p h f", h=H)
                    hm = inter.tile([P, Fh], F32, tag="hm")
                    nc.vector.tensor_add(out=hm[:], in0=r4[:, 0, :],
                                         in1=r4[:, 1, :])
                    hm2 = inter.tile([P, Fh], F32, tag="hm2")
                    nc.vector.tensor_add(out=hm2[:], in0=r4[:, 2, :],
                                         in1=r4[:, 3, :])
                    nc.vector.tensor_add(out=hm[:], in0=hm[:], in1=hm2[:])
                    mx = inter.tile([P, 1], F32, tag="mx")
                    nc.vector.reduce_max(out=mx[:], in_=hm[:],
                                         axis=mybir.AxisListType.X)
                    nmx = inter.tile([P, 1], F32, tag="nmx")
                    nc.vector.tensor_scalar_mul(out=nmx[:], in0=mx[:],
                                                scalar1=-0.25)
                    ex = inter.tile([P, Fh], F32, tag="ex")
                    nc.scalar.activation(ex[:], hm[:], AF.Exp,
                                         bias=nmx[:, :1], scale=0.25)
                    se = inter.tile([P, 1], F32, tag="se")
                    nc.vector.reduce_sum(out=se[:], in_=ex[:],
                                         axis=mybir.AxisListType.X)
                    lnse = inter.tile([P, 1], F32, tag="lnse")
                    nc.scalar.activation(lnse[:], se[:], AF.Ln)
                    ofs = inter.tile([P, 1], F32, tag="ofs")
                    nc.vector.tensor_tensor(out=ofs[:], in0=lnse[:],
                                            in1=nmx[:], op=ALU.subtract)
                    outb = inter.tile([P, Fh], F32, tag="outb")
                    nc.vector.tensor_scalar(
                        out=outb[:], in0=hm[:], scalar1=0.25,
                        scalar2=ofs[:, :1], op0=ALU.mult, op1=ALU.subtract)
                    nc.sync.dma_start(out_t[b * P:(b + 1) * P, :], outb[:])

            if l < 2:
                nc.gpsimd.collective_compute(
                    "AllGather", ALU.bypass,
                    replica_groups=[list(range(NCORES))],
                    ins=[hown_d[l][:].opt()],
                    outs=[hag_d[l][:].opt()])

    nc.compile()
    return nc


# ------------------------------------------------------------------- driver

_CACHE = {}


def _get_program(meta):
    key = tuple(sorted((k, v) for k, v in meta.items()))
    if key not in _CACHE:
        _CACHE[key] = _build(meta)
    return _CACHE[key]


def kernel(x, src, dst, params):
    meta, shared, per_core = _prep(x, src, dst, params)
    nc = _get_program(meta)
    in_maps = [dict(shared, **pc) for pc in per_core]
    res = run_bass_kernel_spmd(nc, in_maps, core_ids=list(range(NCORES)),
                               trace=False)
    N = meta["N"]
    full = np.concatenate([r["out"] for r in res.results], axis=0)
    return full[:N].astype(np.float32)


def bench(x, src, dst, params, iters=20):
    """Median wall-clock (ns) of the NEFF execution via a persistent jitted
    callable with device-resident inputs (input transfer excluded)."""
    import jax
    from jax.sharding import Mesh, PartitionSpec
    from jax.experimental.shard_map import shard_map
    from concourse import bass2jax, mybir as mb

    meta, shared, per_core = _prep(x, src, dst, params)
    nc = _get_program(meta)
    in_maps = [dict(shared, **pc) for pc in per_core]
    bass2jax.install_neuronx_cc_hook()

    part_name = (nc.partition_id_tensor.name if nc.partition_id_tensor
                 else None)
    in_names, out_names, out_avals = [], [], []
    for alloc in nc.m.functions[0].allocations:
        if not isinstance(alloc, mb.MemoryLocationSet):
            continue
        name = alloc.memorylocations[0].name
        if alloc.kind == "ExternalInput":
            if name != part_name:
                in_names.append(name)
        elif alloc.kind == "ExternalOutput":
            out_names.append(name)
            out_avals.append(jax.core.ShapedArray(
                tuple(alloc.tensor_shape), mb.dt.np(alloc.dtype)))
    n_params = len(in_names)
    n_outs = len(out_names)
    all_names = in_names + out_names
    if part_name is not None:
        all_names = all_names + [part_name]

    def _body(*args):
        operands = list(args)
        if part_name is not None:
            operands.append(bass2jax.partition_id_tensor())
        outs = bass2jax._bass_exec_p.bind(
            *operands,
            out_avals=tuple(out_avals),
            in_names=tuple(all_names),
            out_names=tuple(out_names),
            lowering_input_output_aliases=(),
            sim_require_finite=True,
            sim_require_nnan=True,
            nc=nc)
        return tuple(outs)

    devices = jax.devices()[:NCORES]
    mesh = Mesh(np.asarray(devices), ("core",))
    spec = (PartitionSpec("core"),) * (n_params + n_outs)
    sharded = jax.jit(
        shard_map(_body, mesh=mesh, in_specs=spec,
                  out_specs=(PartitionSpec("core"),) * n_outs,
                  check_rep=False),
        donate_argnums=tuple(range(n_params, n_params + n_outs)),
        keep_unused=True)

    concat_in = [np.concatenate([np.asarray(m[nm]) for m in in_maps], axis=0)
                 for nm in in_names]
    sh = jax.sharding.NamedSharding(mesh, PartitionSpec("core"))
    dev_in = [jax.device_put(a, sh) for a in concat_in]

    def zeros():
        return [np.zeros((NCORES * av.shape[0], *av.shape[1:]), av.dtype)
                for av in out_avals]

    # warmup (compile)
    out = sharded(*dev_in, *[jax.device_put(z, sh) for z in zeros()])
    jax.block_until_ready(out)

    times = []
    for _ in range(iters):
        zo = [jax.device_put(z, sh) for z in zeros()]
        jax.block_until_ready(zo)
        t0 = time.perf_counter()
        out = sharded(*dev_in, *zo)
        jax.block_until_ready(out)
        times.append(time.perf_counter() - t0)
    times.sort()
    med = times[len(times) // 2]
    print("bench walls (s):", " ".join(f"{t:.4f}" for t in times[:8]), "...")
    return med * 1e9


# ------------------------------------------------- numpy reference (testing)

def np_reference(x, src, dst, params):
    x = np.asarray(x, np.float64)
    src = np.asarray(src, np.int64)
    dst = np.asarray(dst, np.int64)
    N = x.shape[0]

    def lrelu(v):
        return np.where(v > 0, v, SLOPE * v)

    h = x
    for l, p in enumerate(params):
        f = np.asarray(p["Ws"]).shape[0] // H
        Ws, Wd, Wr = (np.asarray(p[k], np.float64) for k in ("Ws", "Wd", "Wr"))
        bs, bd, br = (np.asarray(p[k], np.float64) for k in ("bs", "bd", "br"))
        attn = np.asarray(p["attn"], np.float64)
        fs = (h @ Ws.T + bs).reshape(N, H, f)
        fd = (h @ Wd.T + bd).reshape(N, H, f)
        e = lrelu(fs[src] + fd[dst])
        score = np.einsum("ehf,hf->eh", e, attn)
        m = np.full((N, H), -np.inf)
        np.maximum.at(m, dst, score)
        s = np.exp(score - m[dst])
        z = np.zeros((N, H))
        np.add.at(z, dst, s)
        a = s / z[dst]
        agg = np.zeros((N, H, f))
        np.add.at(agg, dst, fs[src] * a[:, :, None])
        res = (h @ Wr.T + br).reshape(N, H, f)
        out = agg + res
        h = out.reshape(N, H * f) if l < 2 else out
    h = h.mean(axis=1)
    h = h - h.max(-1, keepdims=True)
    return (h - np.log(np.exp(h).sum(-1, keepdims=True))).astype(np.float32)
